# revision 11
# baseline (speedup 1.0000x reference)
"""DANet DABlock (dual attention) Trainium2 Bass kernel.

Sharding: 8 cores = 4 batch elements x 2 branch roles (PAM / CAM).
Every core runs the SAME program (SPMD): conv1(3x3, Cin->512) + BN + ReLU,
then BOTH attention modules on its own feature map, blended with per-core
gamma masks (PAM cores get gamma_cam_eff=0 and vice versa), then conv2(3x3),
then fused heads (c1 w6/w7 stacked with c1 w8 partial). The host sums the
two w8 partials per batch to form sasc_output.

conv1/conv2 use 1-D Winograd F(2,3) along W (4 transformed points instead
of 6 MACs per 2 outputs -> 1.5x fewer tensor-engine cycles). conv1 GEMMs
run in float32r (full PE rate at free-dim 512), conv2 in bf16. The input
transform (B^T d: 1 add per element) and output transform (A^T m) run on
the vector/gpsimd engines, overlapped with the PE.
"""
import sys
import os
import numpy as np

sys.path.insert(0, '/opt/trn_rl_repo')

import concourse.bass as bass  # noqa: E402
import concourse.mybir as mybir  # noqa: E402
import concourse.tile as tile  # noqa: E402
from concourse import bacc  # noqa: E402
from concourse.masks import make_identity  # noqa: E402

P = 128
F32 = mybir.dt.float32
F32R = mybir.dt.float32r
BF16 = mybir.dt.bfloat16
AF = mybir.ActivationFunctionType
ALU = mybir.AluOpType
AX = mybir.AxisListType

H = W = 64
N = H * W            # 4096 spatial positions
INNER = 512          # feature channels
D = 64               # q/k dim
NT = N // 512        # 8 spatial tiles of 512
CS = INNER // P      # 4 channel subtiles of the 512-dim feature
MS = N // P          # 32 m-subtiles of the 4096 attention positions
NH = 38              # stacked head rows (19 + 19)

# Winograd F(2,3) along W
PTS = 4              # transformed points per tile
TW = W // 2          # 32 tiles along W
BR = 16              # band rows (4 bands)
NB = H // BR
WR = BR + 2          # window rows incl halo


def build(cin=2048, debug=False):
    kq_n = cin // 256                # conv1 k-chunks of 256 channels
    csq = 2                          # 128-ch subtiles per chunk
    nc = bacc.Bacc(None, target_bir_lowering=False, debug=debug)

    # ---------------- inputs ----------------
    xb = nc.dram_tensor("xb", [kq_n, P, csq, H, W], F32R, kind="ExternalInput")
    w1t = nc.dram_tensor("w1t", [kq_n, P, 3 * PTS, csq, INNER], F32R,
                         kind="ExternalInput")
    bn1s = nc.dram_tensor("bn1s", [CS, P], F32, kind="ExternalInput")
    bn1b = nc.dram_tensor("bn1b", [CS, P], F32, kind="ExternalInput")
    wqT = nc.dram_tensor("wqT", [CS, P, D], F32R, kind="ExternalInput")
    wkT = nc.dram_tensor("wkT", [CS, P, D], F32R, kind="ExternalInput")
    bqT = nc.dram_tensor("bqT", [1, D], F32R, kind="ExternalInput")
    bkT = nc.dram_tensor("bkT", [1, D], F32R, kind="ExternalInput")
    wvT = nc.dram_tensor("wvT", [CS, P, INNER], F32R, kind="ExternalInput")
    bvT = nc.dram_tensor("bvT", [1, INNER], F32R, kind="ExternalInput")
    w2t = nc.dram_tensor("w2t", [P, 3 * PTS, CS, INNER], F32,
                         kind="ExternalInput")
    bn2s = nc.dram_tensor("bn2s", [CS, P], F32, kind="ExternalInput")
    bn2b = nc.dram_tensor("bn2b", [CS, P], F32, kind="ExternalInput")
    whT = nc.dram_tensor("whT", [CS, P, NH], F32, kind="ExternalInput")
    hbias = nc.dram_tensor("hbias", [P, 1], F32, kind="ExternalInput")
    gpam = nc.dram_tensor("gpam", [P, 1], F32, kind="ExternalInput")
    gcam = nc.dram_tensor("gcam", [P, 1], F32, kind="ExternalInput")

    oh = nc.dram_tensor("oh", [NH, N], F32, kind="ExternalOutput")

    def win_rows(rg):
        """Clip an 18-row input window (rows rg*16-1 .. rg*16+16) to [0, H)."""
        y0 = rg * BR
        lo = max(0, y0 - 1)
        hi = min(H, y0 + BR + 1)
        return lo, hi, lo - (y0 - 1)

    def wino_in(u, xw, shape):
        """1-D F(2,3) input transform along W: 4 strided adds."""
        nc.vector.tensor_sub(u[:, :, :, 0], xw[:, :, :, 0:W:2],
                             xw[:, :, :, 2:W + 2:2])
        nc.vector.tensor_add(u[:, :, :, 1], xw[:, :, :, 1:W + 1:2],
                             xw[:, :, :, 2:W + 2:2])
        nc.gpsimd.tensor_sub(u[:, :, :, 2], xw[:, :, :, 2:W + 2:2],
                             xw[:, :, :, 1:W + 1:2])
        nc.gpsimd.tensor_sub(u[:, :, :, 3], xw[:, :, :, 1:W + 1:2],
                             xw[:, :, :, 3:W + 2:2])

    with tile.TileContext(nc) as tc:
        with tc.tile_pool(name="const", bufs=1) as cst, \
             tc.tile_pool(name="dram", bufs=1, space="DRAM") as dram:
            ident32 = cst.tile([P, P], F32)
            make_identity(nc, ident32)
            ident = cst.tile([P, P], F32R)
            nc.vector.tensor_copy(ident, ident32)
            ones_bf = cst.tile([P, 1], BF16)
            nc.any.memset(ones_bf, 1.0)
            ones32 = cst.tile([1, INNER], F32)
            nc.any.memset(ones32, 1.0)
            onesw = cst.tile([1, INNER], F32R)
            nc.vector.tensor_copy(onesw, ones32)
            zc = cst.tile([P, 1], F32)
            nc.any.memset(zc, 0.0)
            cshift = cst.tile([P, 1], F32)
            nc.any.memset(cshift, -40.0)
            bn1s_sb = cst.tile([P, CS], F32)
            nc.sync.dma_start(bn1s_sb, bn1s[:].rearrange("s p -> p s"))
            bn1b_sb = cst.tile([P, CS], F32)
            nc.sync.dma_start(bn1b_sb, bn1b[:].rearrange("s p -> p s"))
            bn2s_sb = cst.tile([P, CS], F32)
            nc.sync.dma_start(bn2s_sb, bn2s[:].rearrange("s p -> p s"))
            bn2b_sb = cst.tile([P, CS], F32)
            nc.sync.dma_start(bn2b_sb, bn2b[:].rearrange("s p -> p s"))
            wqT_sb = cst.tile([P, CS, D], F32R)
            nc.sync.dma_start(wqT_sb, wqT[:].rearrange("s p d -> p s d"))
            wkT_sb = cst.tile([P, CS, D], F32R)
            nc.sync.dma_start(wkT_sb, wkT[:].rearrange("s p d -> p s d"))
            bqT_sb = cst.tile([1, D], F32R)
            nc.sync.dma_start(bqT_sb, bqT[:])
            bkT_sb = cst.tile([1, D], F32R)
            nc.sync.dma_start(bkT_sb, bkT[:])
            wvT_sb = cst.tile([P, CS, INNER], F32R)
            nc.sync.dma_start(wvT_sb, wvT[:].rearrange("s p m -> p s m"))
            bvT_sb = cst.tile([1, INNER], F32R)
            nc.sync.dma_start(bvT_sb, bvT[:])
            whT_f = cst.tile([P, CS, NH], F32)
            nc.sync.dma_start(whT_f, whT[:].rearrange("s p m -> p s m"))
            whT_sb = cst.tile([P, CS, NH], BF16)
            nc.vector.tensor_copy(whT_sb, whT_f)
            hbias_sb = cst.tile([P, 1], F32)
            nc.sync.dma_start(hbias_sb, hbias[:])
            gpam_sb = cst.tile([P, 1], F32)
            nc.sync.dma_start(gpam_sb, gpam[:])
            gcam_sb = cst.tile([P, 1], F32)
            nc.sync.dma_start(gcam_sb, gcam[:])

            feat_dram = dram.tile([CS, P, N], F32)
            featbn_dram = dram.tile([CS, P, N], F32R)
            pout_dram = dram.tile([CS, P, N], F32)
            blend_dram = dram.tile([CS, P, H, W], F32R)

            # ====== conv1 (Winograd F(2,3) along W) -> feat_dram ======
            # 512-ch pairs share one PSUM accumulation group, halving the
            # DRAM accumulate round-trips and output-transform passes.
            with nc.named_scope("conv1"), \
                 tc.tile_pool(name="c1", bufs=1) as c1p, \
                 tc.tile_pool(name="c1ps", bufs=8, space="PSUM") as c1ps:
                for kqp in range(kq_n // 2):
                    wpair = []
                    for h in range(2):
                        w1sb = c1p.tile([P, 3 * PTS, csq, INNER], F32R,
                                        tag="w1", bufs=2, name=f"w1sb{h}")
                        nc.sync.dma_start(w1sb, w1t[2 * kqp + h])
                        wpair.append(w1sb)
                    for rg in range(NB):
                        lo, hi, r0 = win_rows(rg)
                        upair = []
                        for h in range(2):
                            xw = c1p.tile([P, csq, WR, W + 2], F32R, tag="xw",
                                          bufs=2, name=f"xw{h}")
                            nc.vector.tensor_copy(
                                xw[:, :, :, 0:1],
                                zc.to_broadcast([P, csq, WR, 1]))
                            nc.vector.tensor_copy(
                                xw[:, :, :, W + 1:W + 2],
                                zc.to_broadcast([P, csq, WR, 1]))
                            if rg == 0:
                                nc.vector.tensor_copy(
                                    xw[:, :, 0:1, :],
                                    zc.to_broadcast([P, csq, 1, W + 2]))
                            if rg == NB - 1:
                                nc.vector.tensor_copy(
                                    xw[:, :, WR - 1:WR, :],
                                    zc.to_broadcast([P, csq, 1, W + 2]))
                            for cs_i in range(csq):
                                nc.sync.dma_start(
                                    xw[:, cs_i, r0:r0 + (hi - lo), 1:W + 1],
                                    xb[2 * kqp + h][:, cs_i, lo:hi, :])
                            u = c1p.tile([P, csq, WR, PTS, TW], F32R, tag="u",
                                         bufs=2, name=f"u{h}")
                            wino_in(u, xw, None)
                            upair.append(u)
                        for ot in range(CS):
                            mt = [c1ps.tile([P, BR, TW], F32, tag="c1",
                                            name=f"m{pi}")
                                  for pi in range(PTS)]
                            for pt_i in range(PTS):
                                i = 0
                                for h in range(2):
                                    for dy in range(3):
                                        for cs_i in range(csq):
                                            nc.tensor.matmul(
                                                mt[pt_i],
                                                wpair[h][:, pt_i * 3 + dy, cs_i,
                                                         ot * P:(ot + 1) * P],
                                                upair[h][:, cs_i, dy:dy + BR,
                                                         pt_i],
                                                start=(i == 0),
                                                stop=(i == 6 * csq - 1))
                                            i += 1
                            st = c1p.tile([P, BR, W], F32, tag="c1ev", bufs=4)
                            t01 = c1p.tile([P, BR, TW], F32, tag="t01", bufs=2)
                            t23 = c1p.tile([P, BR, TW], F32, tag="t23", bufs=2)
                            m1s = c1p.tile([P, BR, TW], F32, tag="m1s", bufs=2)
                            m2s = c1p.tile([P, BR, TW], F32, tag="m2s", bufs=2)
                            # <=1 PSUM operand per op: stage m1/m2 via scalar
                            nc.scalar.activation(m1s, mt[1], AF.Identity)
                            nc.scalar.activation(m2s, mt[2], AF.Identity)
                            nc.vector.tensor_add(t01, mt[0], m1s)
                            nc.gpsimd.tensor_add(st[:, :, 0:W:2], t01, m2s)
                            nc.gpsimd.tensor_sub(t23, m1s, m2s)
                            nc.vector.tensor_sub(st[:, :, 1:W:2], t23, mt[3])
                            nc.gpsimd.dma_start(
                                feat_dram[ot, :, rg * BR * W:(rg + 1) * BR * W]
                                .rearrange("p (r c) -> p r c", r=BR),
                                st,
                                accum_op=(ALU.bypass if kqp == 0 else ALU.add))

            # ===== feat = relu(bn(feat_raw)) -> SBUF + featbn_dram ==========
            with nc.named_scope("pam"), tc.tile_pool(name="qkv", bufs=1) as qkv:
                q_sb = qkv.tile([P, NT, 512], F32R, tag="q")
                k_sb = qkv.tile([P, NT, 512], F32R, tag="k")
                vT = qkv.tile([P, MS, INNER], BF16, tag="vT")
                with tc.tile_pool(name="featp", bufs=1) as featp:
                    feat = featp.tile([P, CS, N], F32R)
                    with tc.tile_pool(name="bnp", bufs=2) as bnp:
                        for ot in range(CS):
                            fst = bnp.tile([P, N], F32, tag="fst")
                            nc.sync.dma_start(fst, feat_dram[ot])
                            nc.scalar.activation(
                                feat[:, ot, :], fst, AF.Relu,
                                bias=bn1b_sb[:, ot:ot + 1],
                                scale=bn1s_sb[:, ot:ot + 1])
                            nc.sync.dma_start(featbn_dram[ot], feat[:, ot, :])

                    # ----- q, k, vT -----------------------------------------
                    with tc.tile_pool(name="qkps", bufs=4, space="PSUM") as qkps:
                        for nt in range(NT):
                            pq = qkps.tile([P, 512], F32, tag="pq", bufs=2)
                            pk = qkps.tile([P, 512], F32, tag="pk", bufs=2)
                            nc.tensor.matmul(pq[0:D], bqT_sb, onesw,
                                             start=True, stop=False)
                            for cs_i in range(CS):
                                nc.tensor.matmul(
                                    pq[0:D], wqT_sb[:, cs_i], feat[:, cs_i,
                                    nt * 512:(nt + 1) * 512],
                                    start=False, stop=(cs_i == CS - 1))
                            nc.tensor.matmul(pk[0:D], bkT_sb, onesw,
                                             start=True, stop=False)
                            for cs_i in range(CS):
                                nc.tensor.matmul(
                                    pk[0:D], wkT_sb[:, cs_i], feat[:, cs_i,
                                    nt * 512:(nt + 1) * 512],
                                    start=False, stop=(cs_i == CS - 1))
                            nc.vector.tensor_copy(q_sb[0:D, nt], pq[0:D])
                            nc.vector.tensor_copy(k_sb[0:D, nt], pk[0:D])
                        for ms in range(MS):
                            pv = qkps.tile([P, INNER], F32, tag="pv")
                            nc.tensor.matmul(pv, onesw[:, 0:P], bvT_sb,
                                             start=True, stop=False)
                            for cs_i in range(CS):
                                nc.tensor.matmul(
                                    pv, feat[:, cs_i, ms * P:(ms + 1) * P],
                                    wvT_sb[:, cs_i],
                                    start=False, stop=(cs_i == CS - 1))
                            nc.vector.tensor_copy(vT[:, ms], pv)

                # ----- PAM attention (feat no longer needed) ----------------
                with tc.tile_pool(name="pam", bufs=1) as pam, \
                     tc.tile_pool(name="pamps", bufs=2, space="PSUM") as pamps:
                    for nt in range(NT):
                        # energyT -> PT = exp(e - 40) bf16, batched exp x4
                        PT = pam.tile([P, MS, 512], BF16, tag="PT", bufs=2)
                        psum_s = pamps.tile([1, 512], F32, tag="ps_s", bufs=1)
                        for mj in range(MS // 2):
                            pet2 = pamps.tile([P, 2, 512], F32, tag="pet2",
                                              bufs=2)
                            for j in range(2):
                                ms = mj * 2 + j
                                nc.tensor.matmul(
                                    pet2[:, j],
                                    k_sb[0:D, ms // 4,
                                         (ms % 4) * P:(ms % 4 + 1) * P],
                                    q_sb[0:D, nt], start=True, stop=True)
                            nc.scalar.activation(
                                PT[:, mj * 2:(mj + 1) * 2, :], pet2, AF.Exp,
                                bias=cshift)
                        for ms in range(MS):
                            nc.tensor.matmul(
                                psum_s, ones_bf, PT[:, ms],
                                start=(ms == 0), stop=(ms == MS - 1))
                        srow = pam.tile([1, 512], F32, tag="srow", bufs=2)
                        nc.vector.reciprocal(srow, psum_s)
                        srg = pam.tile([1, 512], F32, tag="srg", bufs=2)
                        nc.vector.tensor_scalar_mul(srg, srow, gpam_sb[0:1])
                        srgb = pam.tile([P, 512], F32, tag="srgb", bufs=2)
                        nc.gpsimd.partition_broadcast(srgb, srg)

                        for ct in range(CS):
                            po = pamps.tile([P, 512], F32, tag="po", bufs=3)
                            for ms in range(MS):
                                nc.tensor.matmul(
                                    po, vT[:, ms, ct * P:(ct + 1) * P],
                                    PT[:, ms],
                                    start=(ms == 0), stop=(ms == MS - 1))
                            pov = pam.tile([P, 512], F32, tag="pov", bufs=2)
                            nc.vector.tensor_mul(pov, po, srgb)
                            nc.sync.dma_start(
                                pout_dram[ct, :, nt * 512:(nt + 1) * 512], pov)

            # ============= CAM ==============================================
            with nc.named_scope("cam"), tc.tile_pool(name="cam", bufs=1) as cam, \
                 tc.tile_pool(name="camps", bufs=2, space="PSUM") as camps:
                feat2 = cam.tile([P, CS, N], F32R, tag="feat2")
                for cs_i in range(CS):
                    nc.sync.dma_start(feat2[:, cs_i], featbn_dram[cs_i])
                attnT = cam.tile([P, CS, INNER], F32R, tag="attnT")
                with tc.tile_pool(name="camT", bufs=1) as camT:
                    featT = camT.tile([P, MS, INNER], F32R, tag="featT")
                    for ms in range(MS):
                        for cs_i in range(CS):
                            ptr = camps.tile([P, P], F32R, tag="ptr")
                            nc.tensor.transpose(
                                ptr, feat2[:, cs_i, ms * P:(ms + 1) * P], ident)
                            nc.vector.tensor_copy(
                                featT[:, ms, cs_i * P:(cs_i + 1) * P], ptr)
                    for ct in range(CS):
                        pce = camps.tile([P, INNER], F32, tag="pce")
                        for ms in range(MS):
                            nc.tensor.matmul(
                                pce, featT[:, ms, ct * P:(ct + 1) * P],
                                featT[:, ms], start=(ms == 0),
                                stop=(ms == MS - 1))
                        mn = cam.tile([P, 1], F32, tag="mn", bufs=2)
                        nc.vector.tensor_reduce(mn, pce, op=ALU.min, axis=AX.X)
                        psc = cam.tile([P, INNER], F32, tag="psc", bufs=2)
                        scol = cam.tile([P, 1], F32, tag="scol", bufs=2)
                        nc.scalar.activation(psc, pce, AF.Exp, bias=mn,
                                             scale=-1.0, accum_out=scol)
                        srec = cam.tile([P, 1], F32, tag="srec", bufs=2)
                        nc.vector.reciprocal(srec, scol)
                        pn = cam.tile([P, INNER], F32R, tag="pn", bufs=2)
                        nc.vector.tensor_scalar_mul(pn, psc, srec)
                        for ds in range(CS):
                            ptr2 = camps.tile([P, P], F32R, tag="ptr2")
                            nc.tensor.transpose(
                                ptr2, pn[:, ds * P:(ds + 1) * P], ident)
                            nc.vector.tensor_copy(
                                attnT[:, ds, ct * P:(ct + 1) * P], ptr2)
                # cam out + blend (nt outer so conv2 row groups unblock early)
                for nt in range(NT):
                    for ct in range(CS):
                        pco = camps.tile([P, 512], F32, tag="pco")
                        for ds in range(CS):
                            nc.tensor.matmul(
                                pco, attnT[:, ds, ct * P:(ct + 1) * P],
                                feat2[:, ds, nt * 512:(nt + 1) * 512],
                                start=(ds == 0), stop=(ds == CS - 1))
                        tmp = cam.tile([P, 512], F32, tag="tmp", bufs=3)
                        nc.vector.scalar_tensor_tensor(
                            tmp, in0=pco, scalar=gcam_sb, in1=feat2[:, ct,
                            nt * 512:(nt + 1) * 512],
                            op0=ALU.mult, op1=ALU.add)
                        pin = cam.tile([P, 512], F32, tag="pin", bufs=3)
                        nc.sync.dma_start(
                            pin, pout_dram[ct, :, nt * 512:(nt + 1) * 512])
                        bl = cam.tile([P, 512], F32R, tag="bl", bufs=3)
                        nc.vector.tensor_add(bl, tmp, pin)
                        nc.sync.dma_start(
                            blend_dram[ct].rearrange("p h w -> p (h w)")
                            [:, nt * 512:(nt + 1) * 512],
                            bl)

            # ====== conv2 (Winograd F(2,3), bf16) + heads ====================
            with nc.named_scope("conv2"), tc.tile_pool(name="c2", bufs=1) as c2p, \
                 tc.tile_pool(name="c2ps", bufs=1, space="PSUM") as c2ps:
                w2sb = c2p.tile([P, 3 * PTS, CS, INNER], BF16, tag="w2")
                for pd in range(3 * PTS):
                    w2f = c2p.tile([P, CS, INNER], F32, tag="w2f", bufs=2)
                    nc.sync.dma_start(w2f, w2t[:, pd])
                    nc.vector.tensor_copy(w2sb[:, pd], w2f)
                bv2 = blend_dram.rearrange("cs p h w -> p cs h w")
                for rg in range(NB):
                    lo, hi, r0 = win_rows(rg)
                    xw2 = c2p.tile([P, CS, WR, W + 2], F32R, tag="xw2", bufs=2)
                    nc.vector.tensor_copy(
                        xw2[:, :, :, 0:1], zc.to_broadcast([P, CS, WR, 1]))
                    nc.vector.tensor_copy(
                        xw2[:, :, :, W + 1:W + 2],
                        zc.to_broadcast([P, CS, WR, 1]))
                    if rg == 0:
                        nc.vector.tensor_copy(
                            xw2[:, :, 0:1, :], zc.to_broadcast([P, CS, 1, W + 2]))
                    if rg == NB - 1:
                        nc.vector.tensor_copy(
                            xw2[:, :, WR - 1:WR, :],
                            zc.to_broadcast([P, CS, 1, W + 2]))
                    for cs_i in range(CS):
                        nc.sync.dma_start(
                            xw2[:, cs_i, r0:r0 + (hi - lo), 1:W + 1],
                            bv2[:, cs_i, lo:hi, :])
                    u2 = c2p.tile([P, CS, WR, PTS, TW], BF16, tag="u2", bufs=2)
                    wino_in(u2, xw2, None)
                    saconv = c2p.tile([P, CS, BR, W], BF16, tag="saconv",
                                      bufs=2)
                    for ot in range(CS):
                        mt = [c2ps.tile([P, BR, TW], F32, tag="c2", bufs=6,
                                        name=f"m2{pi}")
                              for pi in range(PTS)]
                        for pt_i in range(PTS):
                            i = 0
                            for dy in range(3):
                                for cs_i in range(CS):
                                    nc.tensor.matmul(
                                        mt[pt_i],
                                        w2sb[:, pt_i * 3 + dy, cs_i,
                                             ot * P:(ot + 1) * P],
                                        u2[:, cs_i, dy:dy + BR, pt_i],
                                        start=(i == 0),
                                        stop=(i == 3 * CS - 1))
                                    i += 1
                        yraw = c2p.tile([P, BR, W], F32, tag="yraw", bufs=2)
                        t01 = c2p.tile([P, BR, TW], F32, tag="t01b", bufs=2)
                        t23 = c2p.tile([P, BR, TW], F32, tag="t23b", bufs=2)
                        m1s = c2p.tile([P, BR, TW], F32, tag="m1sb", bufs=2)
                        m2s = c2p.tile([P, BR, TW], F32, tag="m2sb", bufs=2)
                        nc.scalar.activation(m1s, mt[1], AF.Identity)
                        nc.scalar.activation(m2s, mt[2], AF.Identity)
                        nc.vector.tensor_add(t01, mt[0], m1s)
                        nc.gpsimd.tensor_add(yraw[:, :, 0:W:2], t01, m2s)
                        nc.gpsimd.tensor_sub(t23, m1s, m2s)
                        nc.vector.tensor_sub(yraw[:, :, 1:W:2], t23, mt[3])
                        nc.scalar.activation(
                            saconv[:, ot], yraw,
                            AF.Relu, bias=bn2b_sb[:, ot:ot + 1],
                            scale=bn2s_sb[:, ot:ot + 1])
                    for half in range(2):
                        ph = c2ps.tile([P, 512], F32, tag="ph", bufs=2)
                        for cs_i in range(CS):
                            nc.tensor.matmul(
                                ph[0:NH], whT_sb[:, cs_i],
                                saconv[:, cs_i, half * 8:(half + 1) * 8, :],
                                start=(cs_i == 0), stop=(cs_i == CS - 1))
                        oht = c2p.tile([P, 512], F32, tag="oht", bufs=2)
                        nc.scalar.activation(oht[0:NH], ph[0:NH], AF.Identity,
                                             bias=hbias_sb[0:NH])
                        nc.sync.dma_start(
                            oh[:, rg * BR * W + half * 512:
                               rg * BR * W + (half + 1) * 512], oht[0:NH])

    nc.compile()
    return nc


_BUILD_CACHE = {}


def get_nc(cin=2048):
    if cin not in _BUILD_CACHE:
        _BUILD_CACHE[cin] = build(cin)
    return _BUILD_CACHE[cin]


EPS = 1e-5

# F(2,3) weight transform (exact in binary fp)
_G23 = np.array([[1, 0, 0], [.5, .5, .5], [.5, -.5, .5], [0, 0, 1]], np.float64)


def _wino_weights(w, kq_n, csq):
    """w [O, C, 3, 3] -> [kq, P, 12, csq, O] f32 with G applied along kx."""
    O, C = w.shape[0], w.shape[1]
    gw = np.einsum('pk,ocdk->pdoc', _G23, np.asarray(w, np.float64))
    # [4 pt, 3 dy, O, C] -> c = kq*(csq*P) + cs*P + p
    gw = gw.reshape(PTS, 3, O, kq_n, csq, P)
    gw = gw.transpose(3, 5, 0, 1, 4, 2).reshape(kq_n, P, 3 * PTS, csq, O)
    return np.ascontiguousarray(gw, dtype=np.float32)


def _prep_core_inputs(x_b, w1, g1, b1, m1, v1, wq, bqv, wk, bkv, wv, bv,
                      w2, g2, b2, m2, v2, wh_a, wh_b, hb, gp, gc, cin):
    kq_n = cin // 256
    csq = 2
    s1 = (g1 / np.sqrt(v1 + EPS)).astype(np.float32)
    bb1 = (b1 - m1 * s1).astype(np.float32)
    s2 = (g2 / np.sqrt(v2 + EPS)).astype(np.float32)
    bb2 = (b2 - m2 * s2).astype(np.float32)
    whT = np.concatenate([wh_a.T, wh_b.T], axis=1).astype(np.float32)  # [512, 38]
    return {
        "xb": np.ascontiguousarray(
            x_b.reshape(kq_n, csq, P, H, W).transpose(0, 2, 1, 3, 4),
            dtype=np.float32),
        "w1t": _wino_weights(w1, kq_n, csq),
        "bn1s": np.ascontiguousarray(s1.reshape(CS, P)),
        "bn1b": np.ascontiguousarray(bb1.reshape(CS, P)),
        "wqT": np.ascontiguousarray(wq.T.reshape(CS, P, D), dtype=np.float32),
        "wkT": np.ascontiguousarray(wk.T.reshape(CS, P, D), dtype=np.float32),
        "bqT": np.ascontiguousarray(bqv.reshape(1, D), dtype=np.float32),
        "bkT": np.ascontiguousarray(bkv.reshape(1, D), dtype=np.float32),
        "wvT": np.ascontiguousarray(wv.T.reshape(CS, P, INNER), dtype=np.float32),
        "bvT": np.ascontiguousarray(bv.reshape(1, INNER), dtype=np.float32),
        "w2t": _wino_weights(w2, 1, CS)[0],
        "bn2s": np.ascontiguousarray(s2.reshape(CS, P)),
        "bn2b": np.ascontiguousarray(bb2.reshape(CS, P)),
        "whT": np.ascontiguousarray(whT.reshape(CS, P, NH)),
        "hbias": np.ascontiguousarray(hb.reshape(P, 1)),
        "gpam": np.full((P, 1), gp, dtype=np.float32),
        "gcam": np.full((P, 1), gc, dtype=np.float32),
    }


def _make_in_maps(inp):
    x = np.asarray(inp["x"], dtype=np.float32)
    B, cin = x.shape[0], x.shape[1]
    gp = float(np.asarray(inp["gamma_pam"]).reshape(-1)[0])
    gc = float(np.asarray(inp["gamma_cam"]).reshape(-1)[0])
    b6 = np.asarray(inp["b6"], np.float32)
    b7 = np.asarray(inp["b7"], np.float32)
    b8 = np.asarray(inp["b8"], np.float32)
    hb_pam = np.zeros(P, np.float32)
    hb_pam[0:19] = b6
    hb_pam[19:38] = b8
    hb_cam = np.zeros(P, np.float32)
    hb_cam[0:19] = b7

    in_maps = []
    for b in range(B):
        for role in range(2):
            if role == 0:   # PAM
                m = _prep_core_inputs(
                    x[b], np.asarray(inp["w5a"], np.float32), inp["g5a"],
                    inp["b5a"], inp["m5a"], inp["v5a"],
                    np.asarray(inp["wq"], np.float32), inp["bq"],
                    np.asarray(inp["wk"], np.float32), inp["bk"],
                    np.asarray(inp["wv"], np.float32), inp["bv"],
                    np.asarray(inp["w51"], np.float32), inp["g51"],
                    inp["b51"], inp["m51"], inp["v51"],
                    np.asarray(inp["w6"], np.float32),
                    np.asarray(inp["w8"], np.float32),
                    hb_pam, gp, 0.0, cin)
            else:           # CAM
                m = _prep_core_inputs(
                    x[b], np.asarray(inp["w5c"], np.float32), inp["g5c"],
                    inp["b5c"], inp["m5c"], inp["v5c"],
                    np.asarray(inp["wq"], np.float32), inp["bq"],
                    np.asarray(inp["wk"], np.float32), inp["bk"],
                    np.asarray(inp["wv"], np.float32), inp["bv"],
                    np.asarray(inp["w52"], np.float32), inp["g52"],
                    inp["b52"], inp["m52"], inp["v52"],
                    np.asarray(inp["w7"], np.float32),
                    np.asarray(inp["w8"], np.float32),
                    hb_cam, 0.0, gc, cin)
            in_maps.append(m)
    return in_maps


def kernel(x, w5a, g5a, b5a, m5a, v5a, w5c, g5c, b5c, m5c, v5c,
           wq, bq, wk, bk, wv, bv, gamma_pam, gamma_cam,
           w51, g51, b51, m51, v51, w52, g52, b52, m52, v52,
           w6, b6, w7, b7, w8, b8):
    from concourse.bass_utils import run_bass_kernel_spmd

    x = np.asarray(x, dtype=np.float32)
    B, cin = x.shape[0], x.shape[1]
    nc = get_nc(cin)
    in_maps = _make_in_maps(dict(
        x=x, w5a=w5a, g5a=g5a, b5a=b5a, m5a=m5a, v5a=v5a,
        w5c=w5c, g5c=g5c, b5c=b5c, m5c=m5c, v5c=v5c,
        wq=wq, bq=bq, wk=wk, bk=bk, wv=wv, bv=bv,
        gamma_pam=gamma_pam, gamma_cam=gamma_cam,
        w51=w51, g51=g51, b51=b51, m51=m51, v51=v51,
        w52=w52, g52=g52, b52=b52, m52=m52, v52=v52,
        w6=w6, b6=b6, w7=w7, b7=b7, w8=w8, b8=b8))

    res = run_bass_kernel_spmd(nc, in_maps, core_ids=list(range(len(in_maps))))

    sa = np.zeros((B, 19, H, W), np.float32)
    sc = np.zeros((B, 19, H, W), np.float32)
    sasc = np.zeros((B, 19, H, W), np.float32)
    for b in range(B):
        oh_a = res.results[2 * b]["oh"]
        oh_c = res.results[2 * b + 1]["oh"]
        sa[b] = oh_a[0:19].reshape(19, H, W)
        sc[b] = oh_c[0:19].reshape(19, H, W)
        sasc[b] = (oh_a[19:38] + oh_c[19:38]).reshape(19, H, W)
    return sasc, sa, sc
